# revision 1
# baseline (speedup 1.0000x reference)
"""Trainium2 Bass kernel for 16-head causal self-attention with RoPE.

Problem: x:[2,2048,2048] -> MHA(wq,wk,wv,wo, causal mask, RoPE) -> [2,2048,2048].

Sharding (8 NeuronCores): core = b*4 + g, where b in {0,1} is the batch
(data parallel) and g in {0..3} is a head group of 4 heads (tensor parallel
over the 16 heads / 2048 channels: group g owns channels [g*512, (g+1)*512)).
Each core:
  phase A: qT/kT = (x @ wq_loc.T).T with RoPE, v = x @ wv_loc.T   (f32r matmuls)
  phase B: per head, scoresT = kT.T-tiles @ qT (transposed scores [sk, sq]),
           exp on ScalarE (scale=1/sqrt(dh) folded in), causal: strictly-upper
           128x512 tiles skipped, diagonal tiles masked by multiply,
           PV and softmax-denominator (ones-matmul) accumulate in PSUM,
           normalization deferred to one reciprocal+mul after PV.
  phase C: partial out = ao @ wo_loc.T  -> DRAM
Host: out[b] = sum of the 4 group partials + bo.
"""

import math
import sys

sys.path.insert(0, "/opt/trn_rl_repo")

import numpy as np

N_CORES = 8
B, S, D = 2, 2048, 2048
H, DH = 16, 128
G = 4                 # head groups (tensor-parallel factor per batch)
HPG = H // (B * G // B)  # noqa — kept simple below
HPG = H // G          # heads per group = 4
CW = HPG * DH         # channels per group = 512
NT = S // 128         # 16 s-tiles
SC = 512              # free-dim chunk (one PSUM bank of fp32)
NQ = S // SC          # 4 s-chunks

_NC_CACHE: dict = {}


def build_attn_nc(iters: int = 1, phases: int = 3):
    """Build + compile the Bass module (same program for all 8 cores)."""
    import concourse.tile as tile
    from concourse import bacc, mybir

    f32 = mybir.dt.float32
    f32r = mybir.dt.float32r
    AF = mybir.ActivationFunctionType
    SCALE = 1.0 / math.sqrt(DH)

    nc = bacc.Bacc("TRN2", target_bir_lowering=False, debug=False,
                   num_devices=N_CORES)

    xT = nc.dram_tensor("xT", [D, S], f32r, kind="ExternalInput").ap()
    wqT = nc.dram_tensor("wqT", [D, CW], f32r, kind="ExternalInput").ap()
    wkT = nc.dram_tensor("wkT", [D, CW], f32r, kind="ExternalInput").ap()
    wvT = nc.dram_tensor("wvT", [D, CW], f32r, kind="ExternalInput").ap()
    woT = nc.dram_tensor("woT", [CW, D], f32r, kind="ExternalInput").ap()
    bqr = nc.dram_tensor("bqr", [HPG, DH, 1], f32, kind="ExternalInput").ap()
    bkr = nc.dram_tensor("bkr", [HPG, DH, 1], f32, kind="ExternalInput").ap()
    bvb = nc.dram_tensor("bvb", [128, CW], f32, kind="ExternalInput").ap()
    cosT = nc.dram_tensor("cosT", [DH, S], f32, kind="ExternalInput").ap()
    sinT = nc.dram_tensor("sinT", [DH, S], f32, kind="ExternalInput").ap()
    PTd = nc.dram_tensor("PTd", [DH, DH], f32r, kind="ExternalInput").ap()
    onesd = nc.dram_tensor("onesd", [128, 128], f32r, kind="ExternalInput").ap()
    mskT = nc.dram_tensor("mskT", [4, 128, SC], f32r, kind="ExternalInput").ap()

    out = nc.dram_tensor("out", [S, D], f32, kind="ExternalOutput").ap()

    ikind = "ExternalOutput" if phases < 2 else "Internal"
    qTd = nc.dram_tensor("qTd", [CW, S], f32r, kind=ikind).ap()
    kTd = nc.dram_tensor("kTd", [CW, S], f32r, kind=ikind).ap()
    vd = nc.dram_tensor("vd", [S, CW], f32r, kind=ikind).ap()

    with tile.TileContext(nc) as tc:
        for it in range(iters):
            with tc.tile_pool(name="const", bufs=1) as cpool:
                pt_sb = cpool.tile([DH, DH], f32r, name=f"pt{it}", tag="pt")
                nc.scalar.dma_start(pt_sb[:], PTd[:])
                bq_sb, bk_sb = [], []
                for ct in range(HPG):
                    tq = cpool.tile([DH, 1], f32, name=f"bq{ct}_{it}",
                                    tag=f"bq{ct}")
                    nc.scalar.dma_start(tq[:], bqr[ct])
                    bq_sb.append(tq)
                    tk = cpool.tile([DH, 1], f32, name=f"bk{ct}_{it}",
                                    tag=f"bk{ct}")
                    nc.scalar.dma_start(tk[:], bkr[ct])
                    bk_sb.append(tk)
                bvb_sb = cpool.tile([128, CW], f32, name=f"bvb{it}", tag="bvb")
                nc.scalar.dma_start(bvb_sb[:], bvb[:])

                # ---------------- phase A: projections + RoPE ----------
                with tc.tile_pool(name="wpool", bufs=1) as wpool, \
                     tc.tile_pool(name="xqpool", bufs=2) as xqpool, \
                     tc.tile_pool(name="cspool", bufs=2) as cspool, \
                     tc.tile_pool(name="prawp", bufs=2) as prawp, \
                     tc.tile_pool(name="workA", bufs=2) as wkp, \
                     tc.tile_pool(name="psA", bufs=4, space="PSUM") as psA, \
                     tc.tile_pool(name="psR", bufs=2, space="PSUM") as psR:
                    w_sb = {}
                    wdr = {}
                    # issue wq loads first so quarter-0 q matmuls start early;
                    # wk/wv loads are issued interleaved with quarter 0 below
                    for nm, dram in (("q", wqT), ("k", wkT), ("v", wvT)):
                        wdr[nm] = dram.rearrange("(n p) c -> n p c", p=128)
                        w_sb[nm] = []
                    xTr = xT.rearrange("(n p) s -> n p s", p=128)
                    xq0 = []
                    for d in range(NT):
                        t = wpool.tile([128, CW], f32r, name=f"wq{d}_{it}",
                                       tag=f"wq{d}")
                        nc.scalar.dma_start(t[:], wdr["q"][d])
                        w_sb["q"].append(t)
                        tx = xqpool.tile([128, SC], f32r,
                                         name=f"xq{d}_0_{it}", tag=f"xq{d}")
                        nc.scalar.dma_start(tx[:], xTr[d][:, 0:SC])
                        xq0.append(tx)

                    def load_w(nm):
                        for d in range(NT):
                            t = wpool.tile([128, CW], f32r,
                                           name=f"w{nm}{d}_{it}",
                                           tag=f"w{nm}{d}")
                            nc.scalar.dma_start(t[:], wdr[nm][d])
                            w_sb[nm].append(t)

                    for qi in range(NQ):
                        S0 = qi * SC
                        if qi == 0:
                            xq = xq0
                        else:
                            xq = []
                            for d in range(NT):
                                t = xqpool.tile([128, SC], f32r,
                                                name=f"xq{d}_{qi}_{it}",
                                                tag=f"xq{d}")
                                nc.scalar.dma_start(t[:], xTr[d][:, S0:S0 + SC])
                                xq.append(t)
                        cos_c = cspool.tile([DH, SC], f32,
                                            name=f"cos{qi}_{it}", tag="cos")
                        nc.scalar.dma_start(cos_c[:], cosT[:, S0:S0 + SC])
                        sin_c = cspool.tile([DH, SC], f32,
                                            name=f"sin{qi}_{it}", tag="sin")
                        nc.scalar.dma_start(sin_c[:], sinT[:, S0:S0 + SC])
                        for nm, bias_sb, outd in (("q", bq_sb, qTd),
                                                  ("k", bk_sb, kTd)):
                            # sub-loop 1: projection matmuls + bias copy
                            # (ct pairs interleaved across two PSUM banks —
                            # back-to-back same-bank accumulation is ~19ns/MM
                            # slower on HW)
                            praws = []
                            for cp in range(0, HPG, 2):
                                psa = psA.tile([128, SC], f32,
                                               name=f"ps{nm}{cp}_{qi}_{it}",
                                               tag="ps")
                                psb = psA.tile([128, SC], f32,
                                               name=f"ps{nm}{cp+1}_{qi}_{it}",
                                               tag="ps")
                                for d in range(NT):
                                    nc.tensor.matmul(
                                        psa[:],
                                        w_sb[nm][d][:, cp * DH:(cp + 1) * DH],
                                        xq[d][:],
                                        start=(d == 0), stop=(d == NT - 1))
                                    nc.tensor.matmul(
                                        psb[:],
                                        w_sb[nm][d][:, (cp + 1) * DH:
                                                     (cp + 2) * DH],
                                        xq[d][:],
                                        start=(d == 0), stop=(d == NT - 1))
                                for ct, ps in ((cp, psa), (cp + 1, psb)):
                                    praw = prawp.tile(
                                        [128, SC], f32r,
                                        name=f"praw{nm}{ct}_{qi}_{it}",
                                        tag=f"praw{ct}")
                                    nc.vector.tensor_scalar_add(
                                        praw[:], ps[:], bias_sb[ct][:])
                                    praws.append(praw)
                            # sub-loop 2: RoPE rotation matmuls (batched so the
                            # PE never waits inline on the ACT bias-copy)
                            for ct in range(HPG):
                                praw = praws[ct]
                                psr = psR.tile([128, SC], f32,
                                               name=f"psr{nm}{ct}_{qi}_{it}",
                                               tag="psr")
                                nc.tensor.matmul(psr[:], pt_sb[:], praw[:],
                                                 start=True, stop=True)
                                m1 = wkp.tile([128, SC], f32,
                                              name=f"m1{nm}{ct}_{qi}_{it}",
                                              tag="m1")
                                nc.vector.tensor_mul(m1[:], praw[:], cos_c[:])
                                m2 = wkp.tile([128, SC], f32,
                                              name=f"m2{nm}{ct}_{qi}_{it}",
                                              tag="m2")
                                nc.vector.tensor_mul(m2[:], psr[:], sin_c[:])
                                pro = wkp.tile([128, SC], f32r,
                                               name=f"pro{nm}{ct}_{qi}_{it}",
                                               tag="pro")
                                nc.vector.tensor_add(pro[:], m1[:], m2[:])
                                nc.sync.dma_start(
                                    outd[ct * DH:(ct + 1) * DH, S0:S0 + SC],
                                    pro[:])
                            if qi == 0 and nm == "q":
                                load_w("k")
                        if qi == 0:
                            load_w("v")
                        for sp in range(0, 4, 2):
                            psa = psA.tile([128, SC], f32,
                                           name=f"psv{sp}_{qi}_{it}",
                                           tag="ps")
                            psb = psA.tile([128, SC], f32,
                                           name=f"psv{sp+1}_{qi}_{it}",
                                           tag="ps")
                            for d in range(NT):
                                nc.tensor.matmul(
                                    psa[:],
                                    xq[d][:, sp * 128:(sp + 1) * 128],
                                    w_sb["v"][d][:],
                                    start=(d == 0), stop=(d == NT - 1))
                                nc.tensor.matmul(
                                    psb[:],
                                    xq[d][:, (sp + 1) * 128:(sp + 2) * 128],
                                    w_sb["v"][d][:],
                                    start=(d == 0), stop=(d == NT - 1))
                            for st, ps in ((sp, psa), (sp + 1, psb)):
                                vt = wkp.tile([128, SC], f32r,
                                              name=f"vt{st}_{qi}_{it}",
                                              tag="vt")
                                nc.vector.tensor_add(vt[:], ps[:], bvb_sb[:])
                                nc.sync.dma_start(
                                    vd[S0 + st * 128:S0 + (st + 1) * 128, :],
                                    vt[:])

                # ---------------- phase B: attention -------------------
                if phases < 2:
                    continue
                with tc.tile_pool(name="aopool", bufs=1) as aopool, \
                     tc.tile_pool(name="wopool", bufs=1) as wopool, \
                     tc.tile_pool(name="mskpool", bufs=1) as mpool:
                    aoT = aopool.tile([128, HPG * S], f32r, name=f"aoT_{it}",
                                      tag="aoT")
                    ones_sb = mpool.tile([128, 128], f32r, name=f"ones{it}",
                                         tag="ones")
                    nc.sync.dma_start(ones_sb[:], onesd[:])
                    msk_sb = []
                    for rr in range(4):
                        m = mpool.tile([128, SC], f32r, name=f"msk{rr}_{it}",
                                       tag=f"msk{rr}")
                        nc.sync.dma_start(m[:], mskT[rr])
                        msk_sb.append(m)
                    wo_sb = []
                    wor = woT.rearrange("(h p) d -> h p d", p=128)
                    with tc.tile_pool(name="hpool", bufs=2) as hpool, \
                         tc.tile_pool(name="atpool", bufs=8) as atpool, \
                         tc.tile_pool(name="recpool", bufs=2) as recpool, \
                         tc.tile_pool(name="psS", bufs=4, space="PSUM") as psS, \
                         tc.tile_pool(name="psO", bufs=2, space="PSUM") as psO:
                        vdr = vd.rearrange("(t p) c -> t p c", p=128)
                        for h in range(HPG):
                            eng = nc.sync
                            qh_c, kh_c, vh_t = [], [], []
                            for qi in range(NQ):
                                tq = hpool.tile([DH, SC], f32r,
                                                name=f"qh{h}_{qi}_{it}",
                                                tag=f"qh{qi}")
                                eng.dma_start(
                                    tq[:], qTd[h * DH:(h + 1) * DH,
                                               qi * SC:(qi + 1) * SC])
                                qh_c.append(tq)
                                tk = hpool.tile([DH, SC], f32r,
                                                name=f"kh{h}_{qi}_{it}",
                                                tag=f"kh{qi}")
                                eng.dma_start(
                                    tk[:], kTd[h * DH:(h + 1) * DH,
                                               qi * SC:(qi + 1) * SC])
                                kh_c.append(tk)
                                for tt in range(4):
                                    t_ = qi * 4 + tt
                                    tv = hpool.tile([128, DH], f32r,
                                                    name=f"vh{h}_{t_}_{it}",
                                                    tag=f"vh{t_}")
                                    eng.dma_start(
                                        tv[:],
                                        vdr[t_][:, h * DH:(h + 1) * DH])
                                    vh_t.append(tv)
                            for c in range(NQ):
                                q0 = c * SC
                                ntile = 4 * c + 4
                                oT = psO.tile([DH, SC], f32,
                                              name=f"oT{h}{c}_{it}", tag="oT")
                                dn = psO.tile([128, SC], f32,
                                              name=f"dn{h}{c}_{it}", tag="dn")
                                for t_ in range(ntile):
                                    rr0 = t_ - 4 * c
                                    nq0 = rr0 * 128 if rr0 > 0 else 0
                                    sps = psS.tile([128, SC], f32,
                                                   name=f"sps{h}{c}{t_}_{it}",
                                                   tag="sps")
                                    nc.tensor.matmul(
                                        sps[:, nq0:],
                                        kh_c[t_ // 4][:, (t_ % 4) * 128:
                                                      (t_ % 4 + 1) * 128],
                                        qh_c[c][:, nq0:],
                                        start=True, stop=True)
                                    at = atpool.tile([128, SC], f32r,
                                                     name=f"at{h}{c}{t_}_{it}",
                                                     tag="at")
                                    rr = t_ - 4 * c
                                    if rr >= 0:
                                        # diagonal-region tile: columns
                                        # [0, rr*128) are fully masked — skip
                                        # them in exp and in the PV/den
                                        # matmuls; apply the triangular mask
                                        # only to the 128-wide diagonal block
                                        n0 = rr * 128
                                        ate = atpool.tile(
                                            [128, SC], f32r,
                                            name=f"ate{h}{c}{t_}_{it}",
                                            tag="ate")
                                        nc.scalar.activation(
                                            ate[:, n0:], sps[:, n0:],
                                            AF.Exp, bias=0.0, scale=SCALE)
                                        nc.vector.tensor_mul(
                                            at[:, n0:n0 + 128],
                                            ate[:, n0:n0 + 128],
                                            msk_sb[rr][:, n0:n0 + 128])
                                        if n0 + 128 < SC:
                                            nc.vector.tensor_copy(
                                                at[:, n0 + 128:],
                                                ate[:, n0 + 128:])
                                    else:
                                        n0 = 0
                                        nc.scalar.activation(at[:], sps[:],
                                                             AF.Exp, bias=0.0,
                                                             scale=SCALE)
                                    nc.tensor.matmul(
                                        oT[:, n0:],
                                        vh_t[t_][:],
                                        at[:, n0:],
                                        start=(t_ == 0),
                                        stop=(t_ == ntile - 1),
                                        skip_group_check=True)
                                    nc.tensor.matmul(
                                        dn[:, n0:], ones_sb[:], at[:, n0:],
                                        start=(t_ == 0),
                                        stop=(t_ == ntile - 1),
                                        skip_group_check=True)
                                rec = recpool.tile([128, SC], f32,
                                                   name=f"rec{h}{c}_{it}",
                                                   tag="rec")
                                nc.vector.reciprocal(rec[:], dn[:])
                                nc.vector.tensor_mul(
                                    aoT[:, h * S + q0:h * S + q0 + SC],
                                    oT[:], rec[:])
                            if h == 0:
                                # prefetch wo during attention so phase C
                                # starts without a DMA bubble
                                for hh in range(HPG):
                                    t = wopool.tile([128, D], f32r,
                                                    name=f"wo{hh}_{it}",
                                                    tag=f"wo{hh}")
                                    nc.sync.dma_start(t[:], wor[hh])
                                    wo_sb.append(t)

                    # ------------ phase C: output projection ------------
                    if phases < 3:
                        for st in range(4):
                            nc.sync.dma_start(
                                out[st * 128:(st + 1) * 128, :],
                                aoT[:, st * D:(st + 1) * D].bitcast(f32))
                        continue
                    with tc.tile_pool(name="outpool", bufs=3) as outpool, \
                         tc.tile_pool(name="psC", bufs=8, space="PSUM") as psC:
                        for st in range(NT):
                            ops = []
                            for dc in range(4):
                                op = psC.tile([128, SC], f32,
                                              name=f"op{st}{dc}_{it}",
                                              tag="op")
                                ops.append(op)
                            for h in range(HPG):
                                lhs = aoT[:, h * S + st * 128:
                                          h * S + (st + 1) * 128]
                                for dc in range(4):
                                    nc.tensor.matmul(
                                        ops[dc][:], lhs,
                                        wo_sb[h][:, dc * SC:(dc + 1) * SC],
                                        start=(h == 0), stop=(h == HPG - 1))
                            ot = outpool.tile([128, D], f32,
                                              name=f"ot{st}_{it}", tag="ot")
                            for dc in range(4):
                                nc.vector.tensor_copy(
                                    ot[:, dc * SC:(dc + 1) * SC], ops[dc][:])
                                # stream each 512-col slice out as soon as its
                                # copy lands instead of waiting for the full
                                # 2048-col tile
                                nc.sync.dma_start(
                                    out[st * 128:(st + 1) * 128,
                                        dc * SC:(dc + 1) * SC],
                                    ot[:, dc * SC:(dc + 1) * SC])
    nc.compile()
    return nc


def host_prep(inputs: dict) -> list:
    """Build per-core input maps (host-side sharding + relayout)."""
    x = np.asarray(inputs["x"], dtype=np.float32)
    wq = np.asarray(inputs["wq"], dtype=np.float32)
    wk = np.asarray(inputs["wk"], dtype=np.float32)
    wv = np.asarray(inputs["wv"], dtype=np.float32)
    wo = np.asarray(inputs["wo"], dtype=np.float32)
    bq = np.asarray(inputs["bq"], dtype=np.float32)
    bk = np.asarray(inputs["bk"], dtype=np.float32)
    bv = np.asarray(inputs["bv"], dtype=np.float32)
    mask = np.asarray(inputs["mask"])

    inv = 1.0 / (10000.0 ** (np.arange(0, DH, 2, dtype=np.float64) / DH))
    ang = np.arange(S, dtype=np.float64)[:, None] * inv[None, :]
    sin = np.repeat(np.sin(ang), 2, axis=1).astype(np.float32)
    cos = np.repeat(np.cos(ang), 2, axis=1).astype(np.float32)
    cosT = np.ascontiguousarray(cos.T)
    sinT = np.ascontiguousarray(sin.T)

    P = np.zeros((DH, DH), np.float32)
    idx = np.arange(0, DH, 2)
    P[idx, idx + 1] = -1.0    # out[2i]   = -x[2i+1]
    P[idx + 1, idx] = 1.0     # out[2i+1] =  x[2i]
    PT = np.ascontiguousarray(P.T)

    m2 = mask[0, 0]
    mskT = np.zeros((4, 128, SC), np.float32)
    for rr in range(4):
        # keep[i, j] = not masked(q=j, k=rr*128+i)
        mskT[rr] = (~m2[:SC, rr * 128:(rr + 1) * 128]).T.astype(np.float32)

    xTb = [np.ascontiguousarray(x[b].T) for b in range(B)]
    in_maps = []
    for core in range(N_CORES):
        b, g = divmod(core, G)
        c0 = g * CW
        in_maps.append({
            "xT": xTb[b],
            "wqT": np.ascontiguousarray(wq[c0:c0 + CW, :].T),
            "wkT": np.ascontiguousarray(wk[c0:c0 + CW, :].T),
            "wvT": np.ascontiguousarray(wv[c0:c0 + CW, :].T),
            "woT": np.ascontiguousarray(wo[:, c0:c0 + CW].T),
            "bqr": np.ascontiguousarray(
                bq[c0:c0 + CW].reshape(HPG, DH, 1)),
            "bkr": np.ascontiguousarray(
                bk[c0:c0 + CW].reshape(HPG, DH, 1)),
            "bvb": np.ascontiguousarray(
                np.broadcast_to(bv[c0:c0 + CW], (128, CW))),
            "cosT": cosT,
            "sinT": sinT,
            "PTd": PT,
            "onesd": np.ones((128, 128), np.float32),
            "mskT": mskT,
        })
    return in_maps


def _get_nc():
    if "nc" not in _NC_CACHE:
        _NC_CACHE["nc"] = build_attn_nc(iters=1)
    return _NC_CACHE["nc"]


def kernel(**inputs) -> np.ndarray:
    from concourse.bass_utils import run_bass_kernel_spmd

    nc = _get_nc()
    in_maps = host_prep(inputs)
    res = run_bass_kernel_spmd(nc, in_maps, core_ids=list(range(N_CORES)))
    bo = np.asarray(inputs["bo"], dtype=np.float32)
    outp = np.zeros((B, S, D), np.float32)
    for core in range(N_CORES):
        outp[core // G] += res.results[core]["out"]
    outp += bo[None, None, :]
    return outp



# revision 2
# speedup vs baseline: 1.0810x; 1.0810x over previous
"""Trainium2 Bass kernel for 16-head causal self-attention with RoPE (v2).

Problem: x:[2,2048,2048] -> MHA(wq,wk,wv,wo, causal mask, RoPE) -> [2,2048,2048].

Sharding (8 NeuronCores): core = b*4 + g, where b in {0,1} is the batch
(data parallel) and g in {0..3} is a head group of 4 heads (tensor parallel
over the 16 heads / 2048 channels: group g owns channels [g*512, (g+1)*512)).

v2 changes vs v1 (451us):
  - bf16 operands everywhere (tolerance is 2e-2; v1 measured 3.6e-4) -> DMA
    traffic halved, DVE elementwise ops hit the 2x packed mode, diagonal
    attention tiles (free dim 128/256) no longer pay the f32r 4-cycles/row
    penalty.
  - q/k/v stay resident in SBUF between projection and attention (v1 did a
    24MB DRAM roundtrip and a phase barrier).
  - exp batched over two-bank PSUM score groups [128,1024] to amortize the
    ~352-cycle fixed ACT instruction overhead; leading fully-masked columns
    of diagonal groups are skipped.
  - causal mask applied in-place on the bf16 at-tile ([128,128] triangular
    block, DVE 2x) instead of mask-multiply + copy of the whole tile.
  - output partials written as bf16 (host sums in f32 and adds bo).
Each core:
  phase A: qS/kS = RoPE((x @ w.T + b)) [per-head 128 x 2048 SBUF tiles],
           vS = x @ wv.T + bv [16 tiles 128 x 512]
  phase B: per head h, chunk c (512 q): scoresT tiles [k=128, q<=512] ->
           grouped exp (scale=1/sqrt(dh)) -> PV + ones-matmul denominator
           accumulate in PSUM -> reciprocal + mul into aoT (bf16)
  phase C: out partial = ao @ wo_loc.T -> DRAM (bf16)
Host: out[b] = sum of the 4 group partials (f32) + bo.
"""

import math
import sys

sys.path.insert(0, "/opt/trn_rl_repo")

import numpy as np

N_CORES = 8
B, S, D = 2, 2048, 2048
H, DH = 16, 128
G = 4                 # head groups (tensor-parallel factor per batch)
HPG = H // G          # heads per group = 4
CW = HPG * DH         # channels per group = 512
NT = S // 128         # 16 s-tiles
SC = 512              # free-dim chunk (one PSUM bank of fp32)
NQ = S // SC          # 4 s-chunks

_NC_CACHE: dict = {}


def build_attn_nc(iters: int = 1, phases: int = 3):
    """Build + compile the Bass module (same program for all 8 cores)."""
    import concourse.tile as tile
    from concourse import bacc, mybir

    f32 = mybir.dt.float32
    bf16 = mybir.dt.bfloat16
    AF = mybir.ActivationFunctionType
    SCALE = 1.0 / math.sqrt(DH)

    nc = bacc.Bacc("TRN2", target_bir_lowering=False, debug=False,
                   num_devices=N_CORES)

    xT = nc.dram_tensor("xT", [D, S], bf16, kind="ExternalInput").ap()
    wqT = nc.dram_tensor("wqT", [D, CW], bf16, kind="ExternalInput").ap()
    wkT = nc.dram_tensor("wkT", [D, CW], bf16, kind="ExternalInput").ap()
    wvT = nc.dram_tensor("wvT", [D, CW], bf16, kind="ExternalInput").ap()
    woT = nc.dram_tensor("woT", [CW, D], bf16, kind="ExternalInput").ap()
    bqr = nc.dram_tensor("bqr", [HPG, DH, 1], f32, kind="ExternalInput").ap()
    bkr = nc.dram_tensor("bkr", [HPG, DH, 1], f32, kind="ExternalInput").ap()
    bvb = nc.dram_tensor("bvb", [128, CW], f32, kind="ExternalInput").ap()
    cosT = nc.dram_tensor("cosT", [DH, S], bf16, kind="ExternalInput").ap()
    sinT = nc.dram_tensor("sinT", [DH, S], bf16, kind="ExternalInput").ap()
    PTd = nc.dram_tensor("PTd", [DH, DH], bf16, kind="ExternalInput").ap()
    onesd = nc.dram_tensor("onesd", [128, 128], bf16, kind="ExternalInput").ap()
    trid = nc.dram_tensor("trid", [128, 128], bf16, kind="ExternalInput").ap()

    out = nc.dram_tensor("out", [S, D], bf16, kind="ExternalOutput").ap()
    if phases < 2:
        qTd = nc.dram_tensor("qTd", [CW, S], bf16, kind="ExternalOutput").ap()
        kTd = nc.dram_tensor("kTd", [CW, S], bf16, kind="ExternalOutput").ap()
        vd = nc.dram_tensor("vd", [S, CW], bf16, kind="ExternalOutput").ap()

    with tile.TileContext(nc) as tc:
        for it in range(iters):
            with tc.tile_pool(name="const", bufs=1) as cpool, \
                 tc.tile_pool(name="persist", bufs=1) as ppool:
                pt_sb = cpool.tile([DH, DH], bf16, name=f"pt{it}", tag="pt")
                nc.scalar.dma_start(pt_sb[:], PTd[:])
                ones_sb = cpool.tile([128, 128], bf16, name=f"ones{it}",
                                     tag="ones")
                nc.scalar.dma_start(ones_sb[:], onesd[:])
                tri_sb = cpool.tile([128, 128], bf16, name=f"tri{it}",
                                    tag="tri")
                nc.scalar.dma_start(tri_sb[:], trid[:])
                bq_sb, bk_sb = [], []
                for ct in range(HPG):
                    tq = cpool.tile([DH, 1], f32, name=f"bq{ct}_{it}",
                                    tag=f"bq{ct}")
                    nc.scalar.dma_start(tq[:], bqr[ct])
                    bq_sb.append(tq)
                    tk = cpool.tile([DH, 1], f32, name=f"bk{ct}_{it}",
                                    tag=f"bk{ct}")
                    nc.scalar.dma_start(tk[:], bkr[ct])
                    bk_sb.append(tk)
                bvb_sb = cpool.tile([128, CW], f32, name=f"bvb{it}", tag="bvb")
                nc.scalar.dma_start(bvb_sb[:], bvb[:])

                # persistent SBUF tensors (live across phases)
                qS = [ppool.tile([DH, S], bf16, name=f"qS{h}_{it}",
                                 tag=f"qS{h}") for h in range(HPG)]
                kS = [ppool.tile([DH, S], bf16, name=f"kS{h}_{it}",
                                 tag=f"kS{h}") for h in range(HPG)]
                vS = [ppool.tile([128, CW], bf16, name=f"vS{t}_{it}",
                                 tag=f"vS{t}") for t in range(NT)]
                aoT = ppool.tile([128, HPG * S], bf16, name=f"aoT_{it}",
                                 tag="aoT")
                wo_sb = [ppool.tile([128, D], bf16, name=f"wo{h}_{it}",
                                    tag=f"wo{h}") for h in range(HPG)]

                # ---------------- phase A: projections + RoPE ----------
                with tc.tile_pool(name="wpool", bufs=1) as wpool, \
                     tc.tile_pool(name="xqpool", bufs=2) as xqpool, \
                     tc.tile_pool(name="cspool", bufs=2) as cspool, \
                     tc.tile_pool(name="prawp", bufs=2) as prawp, \
                     tc.tile_pool(name="workA", bufs=2) as wkp, \
                     tc.tile_pool(name="psA", bufs=4, space="PSUM") as psA, \
                     tc.tile_pool(name="psR", bufs=2, space="PSUM") as psR:
                    w_sb = {}
                    wdr = {}
                    # issue wq loads first so quarter-0 q matmuls start early;
                    # wk/wv loads are issued interleaved with quarter 0 below
                    for nm, dram in (("q", wqT), ("k", wkT), ("v", wvT)):
                        wdr[nm] = dram.rearrange("(n p) c -> n p c", p=128)
                        w_sb[nm] = []
                    xTr = xT.rearrange("(n p) s -> n p s", p=128)
                    xq0 = []
                    for d in range(NT):
                        t = wpool.tile([128, CW], bf16, name=f"wq{d}_{it}",
                                       tag=f"wq{d}")
                        nc.scalar.dma_start(t[:], wdr["q"][d])
                        w_sb["q"].append(t)
                        tx = xqpool.tile([128, SC], bf16,
                                         name=f"xq{d}_0_{it}", tag=f"xq{d}")
                        nc.scalar.dma_start(tx[:], xTr[d][:, 0:SC])
                        xq0.append(tx)

                    def load_w(nm):
                        for d in range(NT):
                            t = wpool.tile([128, CW], bf16,
                                           name=f"w{nm}{d}_{it}",
                                           tag=f"w{nm}{d}")
                            nc.scalar.dma_start(t[:], wdr[nm][d])
                            w_sb[nm].append(t)

                    for qi in range(NQ):
                        S0 = qi * SC
                        if qi == 0:
                            xq = xq0
                        else:
                            xq = []
                            for d in range(NT):
                                t = xqpool.tile([128, SC], bf16,
                                                name=f"xq{d}_{qi}_{it}",
                                                tag=f"xq{d}")
                                nc.scalar.dma_start(t[:], xTr[d][:, S0:S0 + SC])
                                xq.append(t)
                        cos_c = cspool.tile([DH, SC], bf16,
                                            name=f"cos{qi}_{it}", tag="cos")
                        nc.scalar.dma_start(cos_c[:], cosT[:, S0:S0 + SC])
                        sin_c = cspool.tile([DH, SC], bf16,
                                            name=f"sin{qi}_{it}", tag="sin")
                        nc.scalar.dma_start(sin_c[:], sinT[:, S0:S0 + SC])
                        for nm, bias_sb, outS in (("q", bq_sb, qS),
                                                  ("k", bk_sb, kS)):
                            # sub-loop 1: projection matmuls + bias add
                            # (ct pairs interleaved across two PSUM banks —
                            # back-to-back same-bank accumulation is ~19ns/MM
                            # slower on HW)
                            praws = []
                            for cp in range(0, HPG, 2):
                                psa = psA.tile([128, SC], f32,
                                               name=f"ps{nm}{cp}_{qi}_{it}",
                                               tag="ps")
                                psb = psA.tile([128, SC], f32,
                                               name=f"ps{nm}{cp+1}_{qi}_{it}",
                                               tag="ps")
                                for d in range(NT):
                                    nc.tensor.matmul(
                                        psa[:],
                                        w_sb[nm][d][:, cp * DH:(cp + 1) * DH],
                                        xq[d][:],
                                        start=(d == 0), stop=(d == NT - 1))
                                    nc.tensor.matmul(
                                        psb[:],
                                        w_sb[nm][d][:, (cp + 1) * DH:
                                                     (cp + 2) * DH],
                                        xq[d][:],
                                        start=(d == 0), stop=(d == NT - 1))
                                for ct, ps in ((cp, psa), (cp + 1, psb)):
                                    praw = prawp.tile(
                                        [128, SC], bf16,
                                        name=f"praw{nm}{ct}_{qi}_{it}",
                                        tag=f"praw{ct}")
                                    nc.vector.tensor_scalar_add(
                                        praw[:], ps[:], bias_sb[ct][:])
                                    praws.append(praw)
                            # sub-loop 2: RoPE rotation matmuls (batched so the
                            # PE never waits inline on the bias add)
                            for ct in range(HPG):
                                praw = praws[ct]
                                psr = psR.tile([128, SC], f32,
                                               name=f"psr{nm}{ct}_{qi}_{it}",
                                               tag="psr")
                                nc.tensor.matmul(psr[:], pt_sb[:], praw[:],
                                                 start=True, stop=True)
                                m1 = wkp.tile([128, SC], bf16,
                                              name=f"m1{nm}{ct}_{qi}_{it}",
                                              tag="m1")
                                nc.vector.tensor_mul(m1[:], praw[:], cos_c[:])
                                m2 = wkp.tile([128, SC], bf16,
                                              name=f"m2{nm}{ct}_{qi}_{it}",
                                              tag="m2")
                                nc.vector.tensor_mul(m2[:], psr[:], sin_c[:])
                                nc.vector.tensor_add(
                                    outS[ct][:, S0:S0 + SC], m1[:], m2[:])
                            if qi == 0 and nm == "q":
                                load_w("k")
                        if qi == 0:
                            load_w("v")
                        for sp in range(0, 4, 2):
                            psa = psA.tile([128, SC], f32,
                                           name=f"psv{sp}_{qi}_{it}",
                                           tag="ps")
                            psb = psA.tile([128, SC], f32,
                                           name=f"psv{sp+1}_{qi}_{it}",
                                           tag="ps")
                            for d in range(NT):
                                nc.tensor.matmul(
                                    psa[:],
                                    xq[d][:, sp * 128:(sp + 1) * 128],
                                    w_sb["v"][d][:],
                                    start=(d == 0), stop=(d == NT - 1))
                                nc.tensor.matmul(
                                    psb[:],
                                    xq[d][:, (sp + 1) * 128:(sp + 2) * 128],
                                    w_sb["v"][d][:],
                                    start=(d == 0), stop=(d == NT - 1))
                            for st, ps in ((sp, psa), (sp + 1, psb)):
                                nc.vector.tensor_add(
                                    vS[qi * 4 + st][:], ps[:], bvb_sb[:])

                if phases < 2:
                    for h in range(HPG):
                        nc.sync.dma_start(
                            qTd[h * DH:(h + 1) * DH, :], qS[h][:])
                        nc.sync.dma_start(
                            kTd[h * DH:(h + 1) * DH, :], kS[h][:])
                    for t in range(NT):
                        nc.sync.dma_start(
                            vd[t * 128:(t + 1) * 128, :], vS[t][:])
                    continue

                # ---------------- phase B: attention -------------------
                with tc.tile_pool(name="atpool", bufs=4) as atpool, \
                     tc.tile_pool(name="recpool", bufs=2) as recpool, \
                     tc.tile_pool(name="psS", bufs=2, space="PSUM") as psS, \
                     tc.tile_pool(name="psOo", bufs=2, space="PSUM") as psOo, \
                     tc.tile_pool(name="psOd", bufs=2, space="PSUM") as psOd:
                    for h in range(HPG):
                        qh, kh = qS[h], kS[h]
                        for c in range(NQ):
                            q0 = c * SC
                            ntile = 4 * c + 4
                            oT = psOo.tile([DH, SC], f32,
                                           name=f"oT{h}{c}_{it}", tag="oT")
                            dn = psOd.tile([128, SC], f32,
                                           name=f"dn{h}{c}_{it}", tag="dn")
                            for g in range(ntile // 2):
                                sps = psS.tile([128, 2 * SC], f32,
                                               name=f"sps{h}{c}{g}_{it}",
                                               tag="sps")
                                at = atpool.tile([128, 2 * SC], bf16,
                                                 name=f"at{h}{c}{g}_{it}",
                                                 tag="at")
                                n0s = []
                                for j in range(2):
                                    t_ = 2 * g + j
                                    rr = t_ - 4 * c
                                    n0 = rr * 128 if rr > 0 else 0
                                    n0s.append(n0)
                                    nc.tensor.matmul(
                                        sps[:, j * SC + n0:(j + 1) * SC],
                                        kh[:, t_ * 128:(t_ + 1) * 128],
                                        qh[:, q0 + n0:q0 + SC],
                                        start=True, stop=True,
                                        skip_group_check=True)
                                # one exp over both banks; leading
                                # fully-masked columns of the first tile
                                # are skipped
                                sk = n0s[0]
                                nc.scalar.activation(
                                    at[:, sk:], sps[:, sk:],
                                    AF.Exp, bias=0.0, scale=SCALE)
                                for j in range(2):
                                    t_ = 2 * g + j
                                    rr = t_ - 4 * c
                                    n0 = n0s[j]
                                    if rr >= 0:
                                        # triangular block: in-place mask
                                        nc.vector.tensor_mul(
                                            at[:, j * SC + n0:
                                               j * SC + n0 + 128],
                                            at[:, j * SC + n0:
                                               j * SC + n0 + 128],
                                            tri_sb[:])
                                    nc.tensor.matmul(
                                        oT[:, n0:],
                                        vS[t_][:, h * DH:(h + 1) * DH],
                                        at[:, j * SC + n0:(j + 1) * SC],
                                        start=(t_ == 0),
                                        stop=(t_ == ntile - 1),
                                        skip_group_check=True)
                                    nc.tensor.matmul(
                                        dn[:, n0:], ones_sb[:],
                                        at[:, j * SC + n0:(j + 1) * SC],
                                        start=(t_ == 0),
                                        stop=(t_ == ntile - 1),
                                        skip_group_check=True)
                            rec = recpool.tile([128, SC], f32,
                                               name=f"rec{h}{c}_{it}",
                                               tag="rec")
                            nc.vector.reciprocal(rec[:], dn[:])
                            nc.vector.tensor_mul(
                                aoT[:, h * S + q0:h * S + q0 + SC],
                                oT[:], rec[:])
                        if h == 0:
                            # prefetch wo during attention so phase C
                            # starts without a DMA bubble
                            wor = woT.rearrange("(h p) d -> h p d", p=128)
                            for hh in range(HPG):
                                nc.sync.dma_start(wo_sb[hh][:], wor[hh])

                if phases < 3:
                    for r in range(4):
                        nc.sync.dma_start(
                            out[r * 128:(r + 1) * 128, :],
                            aoT[:, r * D:(r + 1) * D])
                    continue

                # ------------ phase C: output projection ------------
                with tc.tile_pool(name="outpool", bufs=4) as outpool, \
                     tc.tile_pool(name="psC", bufs=8, space="PSUM") as psC:
                    for st in range(NT):
                        ops = []
                        for dc in range(4):
                            op = psC.tile([128, SC], f32,
                                          name=f"op{st}{dc}_{it}",
                                          tag="op")
                            ops.append(op)
                        for hh in range(HPG):
                            lhs = aoT[:, hh * S + st * 128:
                                      hh * S + (st + 1) * 128]
                            for dc in range(4):
                                nc.tensor.matmul(
                                    ops[dc][:], lhs,
                                    wo_sb[hh][:, dc * SC:(dc + 1) * SC],
                                    start=(hh == 0), stop=(hh == HPG - 1))
                        for dc in range(4):
                            ot = outpool.tile([128, SC], bf16,
                                              name=f"ot{st}{dc}_{it}",
                                              tag="ot")
                            # alternate copies between DVE and ACT so
                            # neither engine gates the PSUM drain
                            if dc % 2 == 0:
                                nc.vector.tensor_copy(ot[:], ops[dc][:])
                            else:
                                nc.scalar.activation(ot[:], ops[dc][:],
                                                     AF.Copy)
                            nc.sync.dma_start(
                                out[st * 128:(st + 1) * 128,
                                    dc * SC:(dc + 1) * SC],
                                ot[:])
    nc.compile()
    return nc


def host_prep(inputs: dict) -> list:
    """Build per-core input maps (host-side sharding + relayout)."""
    import ml_dtypes
    bf16 = ml_dtypes.bfloat16

    x = np.asarray(inputs["x"], dtype=np.float32)
    wq = np.asarray(inputs["wq"], dtype=np.float32)
    wk = np.asarray(inputs["wk"], dtype=np.float32)
    wv = np.asarray(inputs["wv"], dtype=np.float32)
    wo = np.asarray(inputs["wo"], dtype=np.float32)
    bq = np.asarray(inputs["bq"], dtype=np.float32)
    bk = np.asarray(inputs["bk"], dtype=np.float32)
    bv = np.asarray(inputs["bv"], dtype=np.float32)
    mask = np.asarray(inputs["mask"])

    inv = 1.0 / (10000.0 ** (np.arange(0, DH, 2, dtype=np.float64) / DH))
    ang = np.arange(S, dtype=np.float64)[:, None] * inv[None, :]
    sin = np.repeat(np.sin(ang), 2, axis=1)
    cos = np.repeat(np.cos(ang), 2, axis=1)
    cosT = np.ascontiguousarray(cos.T).astype(bf16)
    sinT = np.ascontiguousarray(sin.T).astype(bf16)

    P = np.zeros((DH, DH), np.float32)
    idx = np.arange(0, DH, 2)
    P[idx, idx + 1] = -1.0    # out[2i]   = -x[2i+1]
    P[idx + 1, idx] = 1.0     # out[2i+1] =  x[2i]
    PT = np.ascontiguousarray(P.T).astype(bf16)

    m2 = mask[0, 0]
    # keep[k, q] = not masked(q, k) on a diagonal 128 block (same for all)
    tri = np.ascontiguousarray((~m2[:128, :128]).T).astype(bf16)

    xTb = [np.ascontiguousarray(x[b].T).astype(bf16) for b in range(B)]
    in_maps = []
    for core in range(N_CORES):
        b, g = divmod(core, G)
        c0 = g * CW
        in_maps.append({
            "xT": xTb[b],
            "wqT": np.ascontiguousarray(wq[c0:c0 + CW, :].T).astype(bf16),
            "wkT": np.ascontiguousarray(wk[c0:c0 + CW, :].T).astype(bf16),
            "wvT": np.ascontiguousarray(wv[c0:c0 + CW, :].T).astype(bf16),
            "woT": np.ascontiguousarray(wo[:, c0:c0 + CW].T).astype(bf16),
            "bqr": np.ascontiguousarray(
                bq[c0:c0 + CW].reshape(HPG, DH, 1)),
            "bkr": np.ascontiguousarray(
                bk[c0:c0 + CW].reshape(HPG, DH, 1)),
            "bvb": np.ascontiguousarray(
                np.broadcast_to(bv[c0:c0 + CW], (128, CW))),
            "cosT": cosT,
            "sinT": sinT,
            "PTd": PT,
            "onesd": np.ones((128, 128), bf16),
            "trid": tri,
        })
    return in_maps


def _get_nc():
    if "nc" not in _NC_CACHE:
        _NC_CACHE["nc"] = build_attn_nc(iters=1)
    return _NC_CACHE["nc"]


def kernel(**inputs) -> np.ndarray:
    from concourse.bass_utils import run_bass_kernel_spmd

    nc = _get_nc()
    in_maps = host_prep(inputs)
    res = run_bass_kernel_spmd(nc, in_maps, core_ids=list(range(N_CORES)))
    bo = np.asarray(inputs["bo"], dtype=np.float32)
    outp = np.zeros((B, S, D), np.float32)
    for core in range(N_CORES):
        outp[core // G] += np.asarray(res.results[core]["out"],
                                      dtype=np.float32)
    outp += bo[None, None, :]
    return outp


# revision 6
# speedup vs baseline: 1.1066x; 1.0236x over previous
"""Trainium2 Bass kernel for 16-head causal self-attention with RoPE (v2).

Problem: x:[2,2048,2048] -> MHA(wq,wk,wv,wo, causal mask, RoPE) -> [2,2048,2048].

Sharding (8 NeuronCores): core = b*4 + g, where b in {0,1} is the batch
(data parallel) and g in {0..3} is a head group of 4 heads (tensor parallel
over the 16 heads / 2048 channels: group g owns channels [g*512, (g+1)*512)).

v2 changes vs v1 (451us):
  - bf16 operands everywhere (tolerance is 2e-2; v1 measured 3.6e-4) -> DMA
    traffic halved, DVE elementwise ops hit the 2x packed mode, diagonal
    attention tiles (free dim 128/256) no longer pay the f32r 4-cycles/row
    penalty.
  - q/k/v stay resident in SBUF between projection and attention (v1 did a
    24MB DRAM roundtrip and a phase barrier).
  - exp batched over two-bank PSUM score groups [128,1024] to amortize the
    ~352-cycle fixed ACT instruction overhead; leading fully-masked columns
    of diagonal groups are skipped.
  - causal mask applied in-place on the bf16 at-tile ([128,128] triangular
    block, DVE 2x) instead of mask-multiply + copy of the whole tile.
  - output partials written as bf16 (host sums in f32 and adds bo).
Each core:
  phase A: qS/kS = RoPE((x @ w.T + b)) [per-head 128 x 2048 SBUF tiles],
           vS = x @ wv.T + bv [16 tiles 128 x 512]
  phase B: per head h, chunk c (512 q): scoresT tiles [k=128, q<=512] ->
           grouped exp (scale=1/sqrt(dh)) -> PV + ones-matmul denominator
           accumulate in PSUM -> reciprocal + mul into aoT (bf16)
  phase C: out partial = ao @ wo_loc.T -> DRAM (bf16)
Host: out[b] = sum of the 4 group partials (f32) + bo.
"""

import math
import sys

sys.path.insert(0, "/opt/trn_rl_repo")

import numpy as np

N_CORES = 8
B, S, D = 2, 2048, 2048
H, DH = 16, 128
G = 4                 # head groups (tensor-parallel factor per batch)
HPG = H // G          # heads per group = 4
CW = HPG * DH         # channels per group = 512
NT = S // 128         # 16 s-tiles
SC = 512              # free-dim chunk (one PSUM bank of fp32)
NQ = S // SC          # 4 s-chunks

_NC_CACHE: dict = {}


def build_attn_nc(iters: int = 1, phases: int = 3):
    """Build + compile the Bass module (same program for all 8 cores)."""
    import concourse.tile as tile
    from concourse import bacc, mybir

    f32 = mybir.dt.float32
    bf16 = mybir.dt.bfloat16
    AF = mybir.ActivationFunctionType
    SCALE = 1.0 / math.sqrt(DH)

    nc = bacc.Bacc("TRN2", target_bir_lowering=False, debug=False,
                   num_devices=N_CORES)

    xT = nc.dram_tensor("xT", [D, S], bf16, kind="ExternalInput").ap()
    wqT = nc.dram_tensor("wqT", [D, CW], bf16, kind="ExternalInput").ap()
    wkT = nc.dram_tensor("wkT", [D, CW], bf16, kind="ExternalInput").ap()
    wvT = nc.dram_tensor("wvT", [D, CW], bf16, kind="ExternalInput").ap()
    woT = nc.dram_tensor("woT", [CW, D], bf16, kind="ExternalInput").ap()
    bqr = nc.dram_tensor("bqr", [HPG, DH, 1], f32, kind="ExternalInput").ap()
    bkr = nc.dram_tensor("bkr", [HPG, DH, 1], f32, kind="ExternalInput").ap()
    bvb = nc.dram_tensor("bvb", [128, CW], f32, kind="ExternalInput").ap()
    cosT = nc.dram_tensor("cosT", [DH, S], bf16, kind="ExternalInput").ap()
    sinT = nc.dram_tensor("sinT", [DH, S], bf16, kind="ExternalInput").ap()
    PTd = nc.dram_tensor("PTd", [DH, DH], bf16, kind="ExternalInput").ap()
    onesd = nc.dram_tensor("onesd", [128, 128], bf16, kind="ExternalInput").ap()
    trid = nc.dram_tensor("trid", [128, 128], bf16, kind="ExternalInput").ap()

    out = nc.dram_tensor("out", [S, D], bf16, kind="ExternalOutput").ap()
    if phases < 2:
        qTd = nc.dram_tensor("qTd", [CW, S], bf16, kind="ExternalOutput").ap()
        kTd = nc.dram_tensor("kTd", [CW, S], bf16, kind="ExternalOutput").ap()
        vd = nc.dram_tensor("vd", [S, CW], bf16, kind="ExternalOutput").ap()

    with tile.TileContext(nc) as tc:
        for it in range(iters):
            with tc.tile_pool(name="const", bufs=1) as cpool, \
                 tc.tile_pool(name="persist", bufs=1) as ppool:
                pt_sb = cpool.tile([DH, DH], bf16, name=f"pt{it}", tag="pt")
                nc.scalar.dma_start(pt_sb[:], PTd[:])
                ones_sb = cpool.tile([128, 128], bf16, name=f"ones{it}",
                                     tag="ones")
                nc.scalar.dma_start(ones_sb[:], onesd[:])
                tri_sb = cpool.tile([128, 128], bf16, name=f"tri{it}",
                                    tag="tri")
                nc.scalar.dma_start(tri_sb[:], trid[:])
                bq_sb, bk_sb = [], []
                for ct in range(HPG):
                    tq = cpool.tile([DH, 1], f32, name=f"bq{ct}_{it}",
                                    tag=f"bq{ct}")
                    nc.scalar.dma_start(tq[:], bqr[ct])
                    bq_sb.append(tq)
                    tk = cpool.tile([DH, 1], f32, name=f"bk{ct}_{it}",
                                    tag=f"bk{ct}")
                    nc.scalar.dma_start(tk[:], bkr[ct])
                    bk_sb.append(tk)
                bvb_sb = cpool.tile([128, CW], f32, name=f"bvb{it}", tag="bvb")
                nc.scalar.dma_start(bvb_sb[:], bvb[:])

                # persistent SBUF tensors (live across phases)
                qS = [ppool.tile([DH, S], bf16, name=f"qS{h}_{it}",
                                 tag=f"qS{h}") for h in range(HPG)]
                kS = [ppool.tile([DH, S], bf16, name=f"kS{h}_{it}",
                                 tag=f"kS{h}") for h in range(HPG)]
                vS = [ppool.tile([128, CW], bf16, name=f"vS{t}_{it}",
                                 tag=f"vS{t}") for t in range(NT)]
                aoT = ppool.tile([128, HPG * S], bf16, name=f"aoT_{it}",
                                 tag="aoT")
                wo_sb = [ppool.tile([128, D], bf16, name=f"wo{h}_{it}",
                                    tag=f"wo{h}") for h in range(HPG)]

                # ---------------- phase A: projections + RoPE ----------
                with tc.tile_pool(name="wpool", bufs=1) as wpool, \
                     tc.tile_pool(name="xqpool", bufs=2) as xqpool, \
                     tc.tile_pool(name="cspool", bufs=2) as cspool, \
                     tc.tile_pool(name="prawp", bufs=2) as prawp, \
                     tc.tile_pool(name="workA", bufs=2) as wkp, \
                     tc.tile_pool(name="psA", bufs=4, space="PSUM") as psA, \
                     tc.tile_pool(name="psR", bufs=4, space="PSUM") as psR:
                    w_sb = {}
                    wdr = {}
                    # issue wq loads first so quarter-0 q matmuls start early;
                    # wk/wv loads are issued interleaved with quarter 0 below
                    for nm, dram in (("q", wqT), ("k", wkT), ("v", wvT)):
                        wdr[nm] = dram.rearrange("(n p) c -> n p c", p=128)
                        w_sb[nm] = []
                    xTr = xT.rearrange("(n p) s -> n p s", p=128)
                    xq0 = []
                    for d in range(NT):
                        t = wpool.tile([128, CW], bf16, name=f"wq{d}_{it}",
                                       tag=f"wq{d}")
                        nc.scalar.dma_start(t[:], wdr["q"][d])
                        w_sb["q"].append(t)
                        tx = xqpool.tile([128, SC], bf16,
                                         name=f"xq{d}_0_{it}", tag=f"xq{d}")
                        nc.scalar.dma_start(tx[:], xTr[d][:, 0:SC])
                        xq0.append(tx)

                    def load_w(nm):
                        for d in range(NT):
                            t = wpool.tile([128, CW], bf16,
                                           name=f"w{nm}{d}_{it}",
                                           tag=f"w{nm}{d}")
                            nc.scalar.dma_start(t[:], wdr[nm][d])
                            w_sb[nm].append(t)

                    for qi in range(NQ):
                        S0 = qi * SC
                        if qi == 0:
                            xq = xq0
                        else:
                            xq = []
                            for d in range(NT):
                                t = xqpool.tile([128, SC], bf16,
                                                name=f"xq{d}_{qi}_{it}",
                                                tag=f"xq{d}")
                                nc.scalar.dma_start(t[:], xTr[d][:, S0:S0 + SC])
                                xq.append(t)
                        cos_c = cspool.tile([DH, SC], bf16,
                                            name=f"cos{qi}_{it}", tag="cos")
                        nc.scalar.dma_start(cos_c[:], cosT[:, S0:S0 + SC])
                        sin_c = cspool.tile([DH, SC], bf16,
                                            name=f"sin{qi}_{it}", tag="sin")
                        nc.scalar.dma_start(sin_c[:], sinT[:, S0:S0 + SC])
                        for nm, bias_sb, outS in (("q", bq_sb, qS),
                                                  ("k", bk_sb, kS)):
                            # sub-loop 1: projection matmuls + bias add
                            # (ct pairs interleaved across two PSUM banks —
                            # back-to-back same-bank accumulation is ~19ns/MM
                            # slower on HW)
                            praws = []
                            for cp in range(0, HPG, 2):
                                psa = psA.tile([128, SC], f32,
                                               name=f"ps{nm}{cp}_{qi}_{it}",
                                               tag="ps")
                                psb = psA.tile([128, SC], f32,
                                               name=f"ps{nm}{cp+1}_{qi}_{it}",
                                               tag="ps")
                                for d in range(NT):
                                    nc.tensor.matmul(
                                        psa[:],
                                        w_sb[nm][d][:, cp * DH:(cp + 1) * DH],
                                        xq[d][:],
                                        start=(d == 0), stop=(d == NT - 1))
                                    nc.tensor.matmul(
                                        psb[:],
                                        w_sb[nm][d][:, (cp + 1) * DH:
                                                     (cp + 2) * DH],
                                        xq[d][:],
                                        start=(d == 0), stop=(d == NT - 1))
                                for ct, ps in ((cp, psa), (cp + 1, psb)):
                                    praw = prawp.tile(
                                        [128, SC], bf16,
                                        name=f"praw{nm}{ct}_{qi}_{it}",
                                        tag=f"praw{ct}")
                                    nc.vector.tensor_scalar_add(
                                        praw[:], ps[:], bias_sb[ct][:])
                                    praws.append(praw)
                            # sub-loop 2: RoPE rotation matmuls (batched so the
                            # PE never waits inline on the bias add)
                            for ct in range(HPG):
                                praw = praws[ct]
                                psr = psR.tile([128, SC], f32,
                                               name=f"psr{nm}{ct}_{qi}_{it}",
                                               tag="psr")
                                nc.tensor.matmul(psr[:], pt_sb[:], praw[:],
                                                 start=True, stop=True)
                                m1 = wkp.tile([128, SC], bf16,
                                              name=f"m1{nm}{ct}_{qi}_{it}",
                                              tag="m1")
                                nc.vector.tensor_mul(m1[:], praw[:], cos_c[:])
                                m2 = wkp.tile([128, SC], bf16,
                                              name=f"m2{nm}{ct}_{qi}_{it}",
                                              tag="m2")
                                nc.vector.tensor_mul(m2[:], psr[:], sin_c[:])
                                nc.vector.tensor_add(
                                    outS[ct][:, S0:S0 + SC], m1[:], m2[:])
                            if qi == 0 and nm == "q":
                                load_w("k")
                        if qi == 0:
                            load_w("v")
                        for sp in range(0, 4, 2):
                            psa = psA.tile([128, SC], f32,
                                           name=f"psv{sp}_{qi}_{it}",
                                           tag="ps")
                            psb = psA.tile([128, SC], f32,
                                           name=f"psv{sp+1}_{qi}_{it}",
                                           tag="ps")
                            for d in range(NT):
                                nc.tensor.matmul(
                                    psa[:],
                                    xq[d][:, sp * 128:(sp + 1) * 128],
                                    w_sb["v"][d][:],
                                    start=(d == 0), stop=(d == NT - 1))
                                nc.tensor.matmul(
                                    psb[:],
                                    xq[d][:, (sp + 1) * 128:(sp + 2) * 128],
                                    w_sb["v"][d][:],
                                    start=(d == 0), stop=(d == NT - 1))
                            for st, ps in ((sp, psa), (sp + 1, psb)):
                                nc.vector.tensor_add(
                                    vS[qi * 4 + st][:], ps[:], bvb_sb[:])

                if phases < 2:
                    for h in range(HPG):
                        nc.sync.dma_start(
                            qTd[h * DH:(h + 1) * DH, :], qS[h][:])
                        nc.sync.dma_start(
                            kTd[h * DH:(h + 1) * DH, :], kS[h][:])
                    for t in range(NT):
                        nc.sync.dma_start(
                            vd[t * 128:(t + 1) * 128, :], vS[t][:])
                    continue

                # ---------------- phase B: attention -------------------
                with tc.tile_pool(name="atpool", bufs=4) as atpool, \
                     tc.tile_pool(name="recpool", bufs=2) as recpool, \
                     tc.tile_pool(name="psS", bufs=2, space="PSUM") as psS, \
                     tc.tile_pool(name="psOo", bufs=2, space="PSUM") as psOo, \
                     tc.tile_pool(name="psOd", bufs=2, space="PSUM") as psOd:
                    for c in range(NQ):
                        for h in range(HPG):
                            qh, kh = qS[h], kS[h]
                            q0 = c * SC
                            ntile = 4 * c + 4
                            oT = psOo.tile([DH, SC], f32,
                                           name=f"oT{h}{c}_{it}", tag="oT")
                            dn = psOd.tile([128, SC], f32,
                                           name=f"dn{h}{c}_{it}", tag="dn")
                            for g in range(ntile // 2):
                                sps = psS.tile([128, 2 * SC], f32,
                                               name=f"sps{h}{c}{g}_{it}",
                                               tag="sps")
                                at = atpool.tile([128, 2 * SC], bf16,
                                                 name=f"at{h}{c}{g}_{it}",
                                                 tag="at")
                                n0s = []
                                for j in range(2):
                                    t_ = 2 * g + j
                                    rr = t_ - 4 * c
                                    n0 = rr * 128 if rr > 0 else 0
                                    n0s.append(n0)
                                    nc.tensor.matmul(
                                        sps[:, j * SC + n0:(j + 1) * SC],
                                        kh[:, t_ * 128:(t_ + 1) * 128],
                                        qh[:, q0 + n0:q0 + SC],
                                        start=True, stop=True,
                                        skip_group_check=True)
                                # one exp over both banks; leading
                                # fully-masked columns of the first tile
                                # are skipped
                                sk = n0s[0]
                                nc.scalar.activation(
                                    at[:, sk:], sps[:, sk:],
                                    AF.Exp, bias=0.0, scale=SCALE)
                                for j in range(2):
                                    t_ = 2 * g + j
                                    rr = t_ - 4 * c
                                    n0 = n0s[j]
                                    if rr >= 0:
                                        # triangular block: in-place mask
                                        nc.vector.tensor_mul(
                                            at[:, j * SC + n0:
                                               j * SC + n0 + 128],
                                            at[:, j * SC + n0:
                                               j * SC + n0 + 128],
                                            tri_sb[:])
                                    nc.tensor.matmul(
                                        oT[:, n0:],
                                        vS[t_][:, h * DH:(h + 1) * DH],
                                        at[:, j * SC + n0:(j + 1) * SC],
                                        start=(t_ == 0),
                                        stop=(t_ == ntile - 1),
                                        skip_group_check=True)
                                    nc.tensor.matmul(
                                        dn[:, n0:], ones_sb[:],
                                        at[:, j * SC + n0:(j + 1) * SC],
                                        start=(t_ == 0),
                                        stop=(t_ == ntile - 1),
                                        skip_group_check=True)
                            rec = recpool.tile([128, SC], f32,
                                               name=f"rec{h}{c}_{it}",
                                               tag="rec")
                            nc.vector.reciprocal(rec[:], dn[:])
                            nc.vector.tensor_mul(
                                aoT[:, h * S + q0:h * S + q0 + SC],
                                oT[:], rec[:])
                        if c == 0:
                            # prefetch wo during attention so phase C
                            # starts without a DMA bubble
                            wor = woT.rearrange("(h p) d -> h p d", p=128)
                            for hh in range(HPG):
                                nc.sync.dma_start(wo_sb[hh][:], wor[hh])

                if phases < 3:
                    for r in range(4):
                        nc.sync.dma_start(
                            out[r * 128:(r + 1) * 128, :],
                            aoT[:, r * D:(r + 1) * D])
                    continue

                # ------------ phase C: output projection ------------
                with tc.tile_pool(name="outpool", bufs=4) as outpool, \
                     tc.tile_pool(name="psC", bufs=8, space="PSUM") as psC:
                    for st in range(NT):
                        ops = []
                        for dc in range(4):
                            op = psC.tile([128, SC], f32,
                                          name=f"op{st}{dc}_{it}",
                                          tag="op")
                            ops.append(op)
                        for hh in range(HPG):
                            lhs = aoT[:, hh * S + st * 128:
                                      hh * S + (st + 1) * 128]
                            for dc in range(4):
                                nc.tensor.matmul(
                                    ops[dc][:], lhs,
                                    wo_sb[hh][:, dc * SC:(dc + 1) * SC],
                                    start=(hh == 0), stop=(hh == HPG - 1))
                        ot = outpool.tile([128, D], bf16,
                                          name=f"ot{st}_{it}", tag="ot")
                        for dc in range(4):
                            # alternate copies between DVE and ACT so
                            # neither engine gates the PSUM drain
                            if dc % 2 == 0:
                                nc.vector.tensor_copy(
                                    ot[:, dc * SC:(dc + 1) * SC], ops[dc][:])
                            else:
                                nc.scalar.activation(
                                    ot[:, dc * SC:(dc + 1) * SC], ops[dc][:],
                                    AF.Copy)
                        nc.sync.dma_start(
                            out[st * 128:(st + 1) * 128, :], ot[:])
    nc.compile()
    return nc


def host_prep(inputs: dict) -> list:
    """Build per-core input maps (host-side sharding + relayout)."""
    import ml_dtypes
    bf16 = ml_dtypes.bfloat16

    x = np.asarray(inputs["x"], dtype=np.float32)
    wq = np.asarray(inputs["wq"], dtype=np.float32)
    wk = np.asarray(inputs["wk"], dtype=np.float32)
    wv = np.asarray(inputs["wv"], dtype=np.float32)
    wo = np.asarray(inputs["wo"], dtype=np.float32)
    bq = np.asarray(inputs["bq"], dtype=np.float32)
    bk = np.asarray(inputs["bk"], dtype=np.float32)
    bv = np.asarray(inputs["bv"], dtype=np.float32)
    mask = np.asarray(inputs["mask"])

    inv = 1.0 / (10000.0 ** (np.arange(0, DH, 2, dtype=np.float64) / DH))
    ang = np.arange(S, dtype=np.float64)[:, None] * inv[None, :]
    sin = np.repeat(np.sin(ang), 2, axis=1)
    cos = np.repeat(np.cos(ang), 2, axis=1)
    cosT = np.ascontiguousarray(cos.T).astype(bf16)
    sinT = np.ascontiguousarray(sin.T).astype(bf16)

    P = np.zeros((DH, DH), np.float32)
    idx = np.arange(0, DH, 2)
    P[idx, idx + 1] = -1.0    # out[2i]   = -x[2i+1]
    P[idx + 1, idx] = 1.0     # out[2i+1] =  x[2i]
    PT = np.ascontiguousarray(P.T).astype(bf16)

    m2 = mask[0, 0]
    # keep[k, q] = not masked(q, k) on a diagonal 128 block (same for all)
    tri = np.ascontiguousarray((~m2[:128, :128]).T).astype(bf16)

    xTb = [np.ascontiguousarray(x[b].T).astype(bf16) for b in range(B)]
    in_maps = []
    for core in range(N_CORES):
        b, g = divmod(core, G)
        c0 = g * CW
        in_maps.append({
            "xT": xTb[b],
            "wqT": np.ascontiguousarray(wq[c0:c0 + CW, :].T).astype(bf16),
            "wkT": np.ascontiguousarray(wk[c0:c0 + CW, :].T).astype(bf16),
            "wvT": np.ascontiguousarray(wv[c0:c0 + CW, :].T).astype(bf16),
            "woT": np.ascontiguousarray(wo[:, c0:c0 + CW].T).astype(bf16),
            "bqr": np.ascontiguousarray(
                bq[c0:c0 + CW].reshape(HPG, DH, 1)),
            "bkr": np.ascontiguousarray(
                bk[c0:c0 + CW].reshape(HPG, DH, 1)),
            "bvb": np.ascontiguousarray(
                np.broadcast_to(bv[c0:c0 + CW], (128, CW))),
            "cosT": cosT,
            "sinT": sinT,
            "PTd": PT,
            "onesd": np.ones((128, 128), bf16),
            "trid": tri,
        })
    return in_maps


def _get_nc():
    if "nc" not in _NC_CACHE:
        _NC_CACHE["nc"] = build_attn_nc(iters=1)
    return _NC_CACHE["nc"]


def kernel(**inputs) -> np.ndarray:
    from concourse.bass_utils import run_bass_kernel_spmd

    nc = _get_nc()
    in_maps = host_prep(inputs)
    res = run_bass_kernel_spmd(nc, in_maps, core_ids=list(range(N_CORES)))
    bo = np.asarray(inputs["bo"], dtype=np.float32)
    outp = np.zeros((B, S, D), np.float32)
    for core in range(N_CORES):
        outp[core // G] += np.asarray(res.results[core]["out"],
                                      dtype=np.float32)
    outp += bo[None, None, :]
    return outp
